# revision 20
# baseline (speedup 1.0000x reference)
"""Trainium2 Bass kernel for nn_CrossNetwork: 4-layer cross-network.

Reference semantics (per row b of x [B, D], D=512, L=4 layers):
    x_list = [x]
    for i in range(L):
        h = x_list[-1]
        for p in x_list[:-1]:          # sequential dot-product residuals
            s = <h_cur, p>             # scalar per row (h_cur updated each step)
            h_cur = h_cur + s * ones
        y = h_cur @ W[i].T + b[i]
        x_list.append(y)
    out = concat(x_list[1:])           # [B, L*D]

Algebraic restructure (exact): with D_j = <h, p_j> (h = raw layer input)
and sig_j = rowsum(p_j), the accumulated shift S satisfies
    s'_j = D_j + S_{<j} * sig_j ;  S = sum_j s'_j
and since y = (h + S*1) @ W^T + b = h @ W^T + S * wbar + b  (wbar = W.sum(-1)),
the shift never needs to be materialized: it enters as a rank-1 PSUM update.

Dataflow: TRANSPOSED activations. x is host-transposed to x^T [D, B]; all
layers compute y^T[e, b] = sum_d W[e, d] h^T[d, b] with the weight chunk
stationary ([d,e] = W^T chunk) and activations moving -- no PE transposes,
no activation copies for stationarization. Dots <h, p> become elementwise
products (DVE) + partition-dim reduction (ones-stationary matmul). The
bias is folded into the PSUM->SBUF copy (ACT Identity with per-partition
bias). Everything f32r (TF32-like matmul dtype, raw f32 bits in DRAM).

Emission is a diagonal wavefront over (batch-tile, layer) so the in-order
PE queue always has another tile's main matmuls between one tile's
S-dependent instructions.

Sharding: batch split across 8 NeuronCores (data parallel, SPMD).
Output written transposed ([L*D, B] per core); host re-transposes.
"""

import numpy as np

NUM_LAYERS = 4
D = 512
B = 16384
N_CORES = 8
COLS_PER_CORE = B // N_CORES          # 2048 batch columns per core
NB = 512                              # batch columns per tile (moving N)
NBT = COLS_PER_CORE // NB             # 4 batch tiles
NCH = D // 128                        # 4 feature chunks (d and e)

_CACHE = {}


def _build_nc():
    import concourse.tile as tile
    from concourse import bacc, mybir

    F32 = mybir.dt.float32
    F32R = mybir.dt.float32r
    BF16 = mybir.dt.bfloat16
    AF = mybir.ActivationFunctionType
    MUL = mybir.AluOpType.mult
    ADD = mybir.AluOpType.add

    nc = bacc.Bacc("TRN2", target_bir_lowering=False, debug=False)

    XT = nc.dram_tensor("xt", [D, COLS_PER_CORE], BF16, kind="ExternalInput")
    # W pre-tiled on host: WTP[c, p, l*D+e] = W[l, e, c*128+p] so one DMA
    # per d-chunk loads all layers with 4 KiB contiguous lines.
    WTP = nc.dram_tensor("wtp", [NCH, 128, NUM_LAYERS * D], BF16,
                         kind="ExternalInput")
    WBAR = nc.dram_tensor("wbar", [NUM_LAYERS, D], F32R, kind="ExternalInput")
    BIASC = nc.dram_tensor("biasc", [128, NUM_LAYERS * NCH], F32,
                           kind="ExternalInput")
    OUT = nc.dram_tensor("out", [NUM_LAYERS * D, COLS_PER_CORE], BF16,
                         kind="ExternalOutput")

    out_r = OUT.rearrange("(l e p) b -> l e p b", e=NCH, p=128)
    xt_dram = XT.rearrange("(c p) b -> c p b", p=128)

    with tile.TileContext(nc) as tc:
        with (
            tc.tile_pool(name="consts", bufs=1) as consts,
            tc.tile_pool(name="acts", bufs=1) as acts,
            tc.tile_pool(name="prods", bufs=12) as prods,
            tc.tile_pool(name="rows", bufs=2) as rows,
            tc.tile_pool(name="ypsum", bufs=5, space="PSUM") as ypsum,
            tc.tile_pool(name="sigpsum", bufs=1, space="PSUM") as sigpsum,
            tc.tile_pool(name="dotpsum", bufs=2, space="PSUM") as dotpsum,
        ):
            # ---- constants (biasc/wbar DMAs issued after the first-step
            # x/W loads below -- they are first needed ~12us in) ----
            biasc = consts.tile([128, NUM_LAYERS * NCH], F32)
            wbar_sb = consts.tile([1, NUM_LAYERS, D], F32R)
            ones_f = consts.tile([128, 32], F32)
            nc.vector.memset(ones_f[:], 1.0)
            ones32 = consts.tile([128, 32], F32R)
            nc.vector.tensor_copy(ones32[:], ones_f[:])
            ones32b = consts.tile([128, 32], BF16)
            nc.vector.tensor_copy(ones32b[:], ones_f[:])

            xt_sb = consts.tile([128, NCH, COLS_PER_CORE], BF16)
            wt_sb = consts.tile([128, NCH, NUM_LAYERS, D], BF16)

            # whole-chunk loads: 512 KiB per DMA with 4 KiB contiguous
            # lines, chunk 0 first, x and W on opposite hardware-DGE queues
            for c in range(NCH):
                nc.scalar.dma_start(xt_sb[:, c, :], xt_dram[c, :, :])
                nc.sync.dma_start(wt_sb[:, c, :, :], WTP[c, :, :])
            nc.sync.dma_start(biasc[:], BIASC[:, :])
            for i in range(NUM_LAYERS):
                nc.sync.dma_start(wbar_sb[0:1, i, :], WBAR[i:i + 1, :])

            # ---- per-bt state ----
            class St:
                pass

            sts = []
            for bt in range(NBT):
                st = St()
                st.cs = slice(bt * NB, (bt + 1) * NB)
                st.ys = []
                st.sps = {}
                st.S = None
                sts.append(st)
            y_glob = {}

            def x_chunk_of(st):
                return lambda c: xt_sb[:, c, st.cs]

            def prior_chunks(st, i):
                """Chunk getters for the dot priors of layer i's output."""
                return [x_chunk_of(st)] + [
                    (lambda c, t=st.ys[j], cs=st.cs: t[:, c, cs])
                    for j in range(i)]

            def emit_mains(bt, i):
                """16 main matmuls of (bt, layer i) into 4 open psum groups."""
                st = sts[bt]
                h_chunk = x_chunk_of(st) if i == 0 else \
                    (lambda c, t=st.ys[i - 1], cs=st.cs: t[:, c, cs])
                st.yps = [ypsum.tile([128, NB], F32, tag="y", name=f"yp{e}")
                          for e in range(NCH)]
                for c in range(NCH):
                    for e in range(NCH):
                        nc.tensor.matmul(
                            st.yps[e][:], wt_sb[:, c, i, e * 128:(e + 1) * 128],
                            h_chunk(c), start=(c == 0),
                            stop=(c == 3 and i == 0))

            def emit_finish(bt, i):
                """rank-1 shift + bias copies + output DMA of (bt, layer i)."""
                st = sts[bt]
                if i not in y_glob:
                    y_glob[i] = acts.tile([128, NCH, COLS_PER_CORE], BF16,
                                          tag=f"y{i}", name=f"yg{i}")
                yg = y_glob[i]
                for e in range(NCH):
                    yp = st.yps[e]
                    if i >= 1:
                        nc.tensor.matmul(
                            yp[:], wbar_sb[0:1, i, e * 128:(e + 1) * 128],
                            st.S[0:1, :], start=False, stop=True)
                    # bias-folding copy to SBUF, alternating ACT/DVE so the
                    # psum drain runs on two engines
                    bias_ap = biasc[:, i * NCH + e:i * NCH + e + 1]
                    if e % 2 == 1:
                        nc.vector.tensor_scalar(
                            out=yg[:, e, st.cs], in0=yp[:], scalar1=bias_ap,
                            scalar2=None, op0=ADD)
                    else:
                        nc.scalar.activation(
                            yg[:, e, st.cs], yp[:], AF.Identity, bias=bias_ap)
                    # last layer: per-tile DMA as soon as copied (spreads
                    # the final drain); other layers: one wide DMA per
                    # (layer, e-chunk) once all tiles wrote
                    eng = nc.scalar if e % 2 == 0 else nc.sync
                    if i == NUM_LAYERS - 1:
                        eng.dma_start(out_r[i, e, :, st.cs], yg[:, e, st.cs])
                    elif bt == NBT - 1:
                        eng.dma_start(out_r[i, e, :, :], yg[:, e, :])
                st.ys.append(yg)

            def emit_muls(bt, i):
                """DVE products of y_i with all dot priors (early, for lead
                time ahead of the reduce matmuls)."""
                st = sts[bt]
                y_t = st.ys[i]
                st.pend_prods = []
                for p_chunk in prior_chunks(st, i):
                    ps = []
                    for c in range(NCH):
                        prod = prods.tile([128, NB], BF16, tag="prod")
                        nc.vector.tensor_tensor(
                            out=prod[:], in0=y_t[:, c, st.cs],
                            in1=p_chunk(c), op=MUL)
                        ps.append(prod)
                    st.pend_prods.append(ps)

            def emit_reduces(bt, i):
                """sig of y_i (if needed), dot partition-reduces + the row
                recurrence -> S for layer i+1 of tile bt."""
                st = sts[bt]
                y_t = st.ys[i]

                if i in (0, 1):
                    sp_ps = sigpsum.tile([32, NB], F32, tag="sig")
                    for c in range(NCH):
                        nc.tensor.matmul(sp_ps[:], ones32b[:],
                                         y_t[:, c, st.cs],
                                         start=(c == 0), stop=(c == 3))
                    sp = rows.tile([1, NB], F32R, tag=f"sp{i}")
                    nc.scalar.activation(sp[0:1, :], sp_ps[0:1, :],
                                         AF.Copy, bias=1.0)
                    st.sps[i] = sp

                def dot(j):
                    dp = dotpsum.tile([32, NB], F32, tag="dot")
                    for c in range(NCH):
                        nc.tensor.matmul(dp[:], ones32b[:],
                                         st.pend_prods[j][c][:],
                                         start=(c == 0), stop=(c == 3))
                    return dp

                S = rows.tile([1, NB], F32R, tag="S")
                if i == 0:
                    dp0 = dot(0)                        # <y0, x>
                    nc.vector.tensor_copy(S[0:1, :], dp0[0:1, :])
                elif i == 1:
                    dp0 = dot(0)                        # <y1, x>
                    dp1 = dot(1)                        # <y1, y0>
                    # S2 = D_x*(1+sig0) + D_y0
                    t = rows.tile([1, NB], F32R, tag="t")
                    nc.vector.tensor_tensor(
                        out=t[0:1, :], in0=dp0[0:1, :],
                        in1=st.sps[0][0:1, :], op=MUL)
                    nc.vector.tensor_tensor(
                        out=S[0:1, :], in0=dp1[0:1, :],
                        in1=t[0:1, :], op=ADD)
                else:
                    # S3 = (D_x*(1+sig0) + D_y0)*(1+sig1) + D_y1
                    dp0 = dot(0)                        # <y2, x>
                    dp1 = dot(1)                        # <y2, y0>
                    t1 = rows.tile([1, NB], F32R, tag="t")
                    nc.vector.tensor_tensor(
                        out=t1[0:1, :], in0=dp0[0:1, :],
                        in1=st.sps[0][0:1, :], op=MUL)
                    t2 = rows.tile([1, NB], F32R, tag="t")
                    nc.vector.tensor_tensor(
                        out=t2[0:1, :], in0=dp1[0:1, :],
                        in1=t1[0:1, :], op=ADD)
                    dp2 = dot(2)                        # <y2, y1>
                    t3 = rows.tile([1, NB], F32R, tag="t")
                    nc.vector.tensor_tensor(
                        out=t3[0:1, :], in0=t2[0:1, :],
                        in1=st.sps[1][0:1, :], op=MUL)
                    nc.vector.tensor_tensor(
                        out=S[0:1, :], in0=dp2[0:1, :],
                        in1=t3[0:1, :], op=ADD)
                st.S = S
                st.pend_prods = None

            # ---- diagonal wavefront over (bt, layer), deepest layer first.
            # Per position: mains first (always-ready PE work), then the
            # just-in-time reduce matmuls + recurrence of the predecessor
            # step (whose DVE products were emitted ~a full wavefront step
            # earlier), then this step's rank-1 + copies, then its products.
            pending = []
            for diag in range(NBT + NUM_LAYERS - 1):
                for bt in range(min(diag, NBT - 1) + 1):
                    i = diag - bt
                    if not (0 <= i < NUM_LAYERS):
                        continue
                    emit_mains(bt, i)
                    if i >= 1:
                        assert pending[0] == (bt, i - 1)
                        emit_reduces(*pending.pop(0))
                    emit_finish(bt, i)
                    if i < NUM_LAYERS - 1:
                        emit_muls(bt, i)
                        pending.append((bt, i))

    nc.compile()
    return nc


def _host_prep(x, W, b):
    import ml_dtypes
    BF = ml_dtypes.bfloat16
    xT = np.ascontiguousarray(np.asarray(x, np.float32).T.astype(BF))
    # WTP[c, p, l*D+e] = W[l, e, c*128+p]
    WTP = np.ascontiguousarray(
        W.astype(BF).transpose(2, 0, 1)                 # [d, l, e]
        .reshape(NCH, 128, NUM_LAYERS * D))
    wbar = np.ascontiguousarray(W.sum(-1, dtype=np.float32))        # [L, D]
    # bias columns: biasc[p, i*NCH+e] = b[i, e*128+p]
    biasc = np.ascontiguousarray(
        b.reshape(NUM_LAYERS, NCH, 128).transpose(2, 0, 1)
        .reshape(128, NUM_LAYERS * NCH))
    return xT, WTP, wbar, biasc


def run_shards(x, W, b, **spmd_kwargs):
    """Run the SPMD kernel; returns (full_output, BassKernelResults)."""
    from concourse.bass_utils import run_bass_kernel_spmd

    x = np.asarray(x, np.float32)
    W = np.asarray(W, np.float32)
    b = np.asarray(b, np.float32)
    xT, WTP, wbar, biasc = _host_prep(x, W, b)

    if "nc" not in _CACHE:
        _CACHE["nc"] = _build_nc()
    nc = _CACHE["nc"]

    in_maps = []
    for c in range(N_CORES):
        shard = np.ascontiguousarray(
            xT[:, c * COLS_PER_CORE:(c + 1) * COLS_PER_CORE])
        in_maps.append({"xt": shard, "wtp": WTP, "wbar": wbar,
                        "biasc": biasc})

    res = run_bass_kernel_spmd(nc, in_maps, core_ids=list(range(N_CORES)),
                               **spmd_kwargs)
    # per-core out: [L*D, COLS_PER_CORE] transposed; gather + re-transpose
    outT = np.concatenate(
        [np.asarray(r["out"], np.float32) for r in res.results], axis=1)
    out = np.ascontiguousarray(outT.T)                              # [B, L*D]
    return out, res


def kernel(x, W, b):
    out, _ = run_shards(x, W, b)
    return out


# revision 21
# speedup vs baseline: 1.0233x; 1.0233x over previous
"""Trainium2 Bass kernel for nn_CrossNetwork: 4-layer cross-network.

Reference semantics (per row b of x [B, D], D=512, L=4 layers):
    x_list = [x]
    for i in range(L):
        h = x_list[-1]
        for p in x_list[:-1]:          # sequential dot-product residuals
            s = <h_cur, p>             # scalar per row (h_cur updated each step)
            h_cur = h_cur + s * ones
        y = h_cur @ W[i].T + b[i]
        x_list.append(y)
    out = concat(x_list[1:])           # [B, L*D]

Algebraic restructure (exact): with D_j = <h, p_j> (h = raw layer input)
and sig_j = rowsum(p_j), the accumulated shift S satisfies
    s'_j = D_j + S_{<j} * sig_j ;  S = sum_j s'_j
and since y = (h + S*1) @ W^T + b = h @ W^T + S * wbar + b  (wbar = W.sum(-1)),
the shift never needs to be materialized: it enters as a rank-1 PSUM update.

Dataflow: TRANSPOSED activations. x is host-transposed to x^T [D, B]; all
layers compute y^T[e, b] = sum_d W[e, d] h^T[d, b] with the weight chunk
stationary ([d,e] = W^T chunk) and activations moving -- no PE transposes,
no activation copies for stationarization. Dots <h, p> become elementwise
products (DVE) + partition-dim reduction (ones-stationary matmul). The
bias is folded into the PSUM->SBUF copy (ACT Identity with per-partition
bias). Everything f32r (TF32-like matmul dtype, raw f32 bits in DRAM).

Emission is a diagonal wavefront over (batch-tile, layer) so the in-order
PE queue always has another tile's main matmuls between one tile's
S-dependent instructions.

Sharding: batch split across 8 NeuronCores (data parallel, SPMD).
Output written transposed ([L*D, B] per core); host re-transposes.
"""

import numpy as np

NUM_LAYERS = 4
D = 512
B = 16384
N_CORES = 8
COLS_PER_CORE = B // N_CORES          # 2048 batch columns per core
NB = 512                              # batch columns per tile (moving N)
NBT = COLS_PER_CORE // NB             # 4 batch tiles
NCH = D // 128                        # 4 feature chunks (d and e)

_CACHE = {}


def _build_nc():
    import concourse.tile as tile
    from concourse import bacc, mybir

    F32 = mybir.dt.float32
    F32R = mybir.dt.float32r
    BF16 = mybir.dt.bfloat16
    AF = mybir.ActivationFunctionType
    MUL = mybir.AluOpType.mult
    ADD = mybir.AluOpType.add

    nc = bacc.Bacc("TRN2", target_bir_lowering=False, debug=False)

    XT = nc.dram_tensor("xt", [D, COLS_PER_CORE], BF16, kind="ExternalInput")
    # W pre-tiled on host: WTP[c, p, l*D+e] = W[l, e, c*128+p] so one DMA
    # per d-chunk loads all layers with 4 KiB contiguous lines.
    WTP = nc.dram_tensor("wtp", [NCH, 128, NUM_LAYERS * D], BF16,
                         kind="ExternalInput")
    WBAR = nc.dram_tensor("wbar", [NUM_LAYERS, D], F32R, kind="ExternalInput")
    BIASC = nc.dram_tensor("biasc", [128, NUM_LAYERS * NCH], F32,
                           kind="ExternalInput")
    OUT = nc.dram_tensor("out", [NUM_LAYERS * D, COLS_PER_CORE], BF16,
                         kind="ExternalOutput")

    out_r = OUT.rearrange("(l e p) b -> l e p b", e=NCH, p=128)
    xt_dram = XT.rearrange("(c p) b -> c p b", p=128)

    with tile.TileContext(nc) as tc:
        with (
            tc.tile_pool(name="consts", bufs=1) as consts,
            tc.tile_pool(name="acts", bufs=1) as acts,
            tc.tile_pool(name="prods", bufs=12) as prods,
            tc.tile_pool(name="rows", bufs=2) as rows,
            tc.tile_pool(name="ypsum", bufs=5, space="PSUM") as ypsum,
            tc.tile_pool(name="sigpsum", bufs=1, space="PSUM") as sigpsum,
            tc.tile_pool(name="dotpsum", bufs=2, space="PSUM") as dotpsum,
        ):
            # ---- constants (biasc/wbar DMAs issued after the first-step
            # x/W loads below -- they are first needed ~12us in) ----
            biasc = consts.tile([128, NUM_LAYERS * NCH], F32)
            wbar_sb = consts.tile([1, NUM_LAYERS, D], F32R)
            ones_f = consts.tile([128, 32], F32)
            nc.vector.memset(ones_f[:], 1.0)
            ones32 = consts.tile([128, 32], F32R)
            nc.vector.tensor_copy(ones32[:], ones_f[:])
            ones32b = consts.tile([128, 32], BF16)
            nc.vector.tensor_copy(ones32b[:], ones_f[:])

            xt_sb = consts.tile([128, NCH, COLS_PER_CORE], BF16)
            wt_sb = consts.tile([128, NCH, NUM_LAYERS, D], BF16)

            # whole-chunk loads: 512 KiB per DMA with 4 KiB contiguous
            # lines, chunk 0 first, x and W on opposite hardware-DGE queues
            for c in range(NCH):
                nc.scalar.dma_start(xt_sb[:, c, :], xt_dram[c, :, :])
                nc.sync.dma_start(wt_sb[:, c, :, :], WTP[c, :, :])
            nc.sync.dma_start(biasc[:], BIASC[:, :])
            for i in range(NUM_LAYERS):
                nc.sync.dma_start(wbar_sb[0:1, i, :], WBAR[i:i + 1, :])

            # ---- per-bt state ----
            class St:
                pass

            sts = []
            for bt in range(NBT):
                st = St()
                st.cs = slice(bt * NB, (bt + 1) * NB)
                st.ys = []
                st.sps = {}
                st.S = None
                sts.append(st)
            y_glob = {}

            def x_chunk_of(st):
                return lambda c: xt_sb[:, c, st.cs]

            def prior_chunks(st, i):
                """Chunk getters for the dot priors of layer i's output."""
                return [x_chunk_of(st)] + [
                    (lambda c, t=st.ys[j], cs=st.cs: t[:, c, cs])
                    for j in range(i)]

            def emit_mains(bt, i):
                """16 main matmuls of (bt, layer i) into 4 open psum groups."""
                st = sts[bt]
                h_chunk = x_chunk_of(st) if i == 0 else \
                    (lambda c, t=st.ys[i - 1], cs=st.cs: t[:, c, cs])
                st.yps = [ypsum.tile([128, NB], F32, tag="y", name=f"yp{e}")
                          for e in range(NCH)]
                for c in range(NCH):
                    for e in range(NCH):
                        nc.tensor.matmul(
                            st.yps[e][:], wt_sb[:, c, i, e * 128:(e + 1) * 128],
                            h_chunk(c), start=(c == 0),
                            stop=(c == 3 and i == 0))

            def emit_finish(bt, i):
                """rank-1 shift + bias copies + output DMA of (bt, layer i)."""
                st = sts[bt]
                if i not in y_glob:
                    y_glob[i] = acts.tile([128, NCH, COLS_PER_CORE], BF16,
                                          tag=f"y{i}", name=f"yg{i}")
                yg = y_glob[i]
                for e in range(NCH):
                    yp = st.yps[e]
                    if i >= 1:
                        nc.tensor.matmul(
                            yp[:], wbar_sb[0:1, i, e * 128:(e + 1) * 128],
                            st.S[0:1, :], start=False, stop=True)
                    # bias-folding copy to SBUF on ACT (DVE is loaded with
                    # the dot products + pair pre-sums)
                    bias_ap = biasc[:, i * NCH + e:i * NCH + e + 1]
                    nc.scalar.activation(
                        yg[:, e, st.cs], yp[:], AF.Identity, bias=bias_ap)
                    # last layer: per-tile DMA as soon as copied (spreads
                    # the final drain); other layers: one wide DMA per
                    # (layer, e-chunk) once all tiles wrote
                    eng = nc.scalar if e % 2 == 0 else nc.sync
                    if i == NUM_LAYERS - 1:
                        eng.dma_start(out_r[i, e, :, st.cs], yg[:, e, st.cs])
                    elif bt == NBT - 1:
                        eng.dma_start(out_r[i, e, :, :], yg[:, e, :])
                st.ys.append(yg)

            def emit_muls(bt, i):
                """DVE products of y_i with all dot priors (early, for lead
                time ahead of the reduce matmuls)."""
                st = sts[bt]
                y_t = st.ys[i]
                st.pend_prods = []
                for p_chunk in prior_chunks(st, i):
                    ps = []
                    for c in range(NCH):
                        prod = prods.tile([128, NB], BF16, tag="prod")
                        nc.vector.tensor_tensor(
                            out=prod[:], in0=y_t[:, c, st.cs],
                            in1=p_chunk(c), op=MUL)
                        ps.append(prod)
                    pairs = []
                    for h in range(2):
                        psum2 = prods.tile([128, NB], BF16, tag="psum2")
                        nc.vector.tensor_tensor(
                            out=psum2[:], in0=ps[2 * h][:],
                            in1=ps[2 * h + 1][:], op=ADD)
                        pairs.append(psum2)
                    st.pend_prods.append(pairs)

            def emit_reduces(bt, i):
                """sig of y_i (if needed), dot partition-reduces + the row
                recurrence -> S for layer i+1 of tile bt."""
                st = sts[bt]
                y_t = st.ys[i]

                if i in (0, 1):
                    sp_ps = sigpsum.tile([32, NB], F32, tag="sig")
                    for c in range(NCH):
                        nc.tensor.matmul(sp_ps[:], ones32b[:],
                                         y_t[:, c, st.cs],
                                         start=(c == 0), stop=(c == 3))
                    sp = rows.tile([1, NB], F32R, tag=f"sp{i}")
                    nc.scalar.activation(sp[0:1, :], sp_ps[0:1, :],
                                         AF.Copy, bias=1.0)
                    st.sps[i] = sp

                def dot(j):
                    dp = dotpsum.tile([32, NB], F32, tag="dot")
                    for h in range(2):
                        nc.tensor.matmul(dp[:], ones32b[:],
                                         st.pend_prods[j][h][:],
                                         start=(h == 0), stop=(h == 1))
                    return dp

                S = rows.tile([1, NB], F32R, tag="S")
                if i == 0:
                    dp0 = dot(0)                        # <y0, x>
                    nc.vector.tensor_copy(S[0:1, :], dp0[0:1, :])
                elif i == 1:
                    dp0 = dot(0)                        # <y1, x>
                    dp1 = dot(1)                        # <y1, y0>
                    # S2 = D_x*(1+sig0) + D_y0
                    t = rows.tile([1, NB], F32R, tag="t")
                    nc.vector.tensor_tensor(
                        out=t[0:1, :], in0=dp0[0:1, :],
                        in1=st.sps[0][0:1, :], op=MUL)
                    nc.vector.tensor_tensor(
                        out=S[0:1, :], in0=dp1[0:1, :],
                        in1=t[0:1, :], op=ADD)
                else:
                    # S3 = (D_x*(1+sig0) + D_y0)*(1+sig1) + D_y1
                    dp0 = dot(0)                        # <y2, x>
                    dp1 = dot(1)                        # <y2, y0>
                    t1 = rows.tile([1, NB], F32R, tag="t")
                    nc.vector.tensor_tensor(
                        out=t1[0:1, :], in0=dp0[0:1, :],
                        in1=st.sps[0][0:1, :], op=MUL)
                    t2 = rows.tile([1, NB], F32R, tag="t")
                    nc.vector.tensor_tensor(
                        out=t2[0:1, :], in0=dp1[0:1, :],
                        in1=t1[0:1, :], op=ADD)
                    dp2 = dot(2)                        # <y2, y1>
                    t3 = rows.tile([1, NB], F32R, tag="t")
                    nc.vector.tensor_tensor(
                        out=t3[0:1, :], in0=t2[0:1, :],
                        in1=st.sps[1][0:1, :], op=MUL)
                    nc.vector.tensor_tensor(
                        out=S[0:1, :], in0=dp2[0:1, :],
                        in1=t3[0:1, :], op=ADD)
                st.S = S
                st.pend_prods = None

            # ---- diagonal wavefront over (bt, layer), deepest layer first.
            # Per position: mains first (always-ready PE work), then the
            # just-in-time reduce matmuls + recurrence of the predecessor
            # step (whose DVE products were emitted ~a full wavefront step
            # earlier), then this step's rank-1 + copies, then its products.
            pending = []
            for diag in range(NBT + NUM_LAYERS - 1):
                for bt in range(min(diag, NBT - 1) + 1):
                    i = diag - bt
                    if not (0 <= i < NUM_LAYERS):
                        continue
                    emit_mains(bt, i)
                    if i >= 1:
                        assert pending[0] == (bt, i - 1)
                        emit_reduces(*pending.pop(0))
                    emit_finish(bt, i)
                    if i < NUM_LAYERS - 1:
                        emit_muls(bt, i)
                        pending.append((bt, i))

    nc.compile()
    return nc


def _host_prep(x, W, b):
    import ml_dtypes
    BF = ml_dtypes.bfloat16
    xT = np.ascontiguousarray(np.asarray(x, np.float32).T.astype(BF))
    # WTP[c, p, l*D+e] = W[l, e, c*128+p]
    WTP = np.ascontiguousarray(
        W.astype(BF).transpose(2, 0, 1)                 # [d, l, e]
        .reshape(NCH, 128, NUM_LAYERS * D))
    wbar = np.ascontiguousarray(W.sum(-1, dtype=np.float32))        # [L, D]
    # bias columns: biasc[p, i*NCH+e] = b[i, e*128+p]
    biasc = np.ascontiguousarray(
        b.reshape(NUM_LAYERS, NCH, 128).transpose(2, 0, 1)
        .reshape(128, NUM_LAYERS * NCH))
    return xT, WTP, wbar, biasc


def run_shards(x, W, b, **spmd_kwargs):
    """Run the SPMD kernel; returns (full_output, BassKernelResults)."""
    from concourse.bass_utils import run_bass_kernel_spmd

    x = np.asarray(x, np.float32)
    W = np.asarray(W, np.float32)
    b = np.asarray(b, np.float32)
    xT, WTP, wbar, biasc = _host_prep(x, W, b)

    if "nc" not in _CACHE:
        _CACHE["nc"] = _build_nc()
    nc = _CACHE["nc"]

    in_maps = []
    for c in range(N_CORES):
        shard = np.ascontiguousarray(
            xT[:, c * COLS_PER_CORE:(c + 1) * COLS_PER_CORE])
        in_maps.append({"xt": shard, "wtp": WTP, "wbar": wbar,
                        "biasc": biasc})

    res = run_bass_kernel_spmd(nc, in_maps, core_ids=list(range(N_CORES)),
                               **spmd_kwargs)
    # per-core out: [L*D, COLS_PER_CORE] transposed; gather + re-transpose
    outT = np.concatenate(
        [np.asarray(r["out"], np.float32) for r in res.results], axis=1)
    out = np.ascontiguousarray(outT.T)                              # [B, L*D]
    return out, res


def kernel(x, W, b):
    out, _ = run_shards(x, W, b)
    return out
